# revision 12
# baseline (speedup 1.0000x reference)
"""Sparse attention (template/search) Trainium2 Bass kernel.

Reference computation (B=64, N=320, C=768, H=12, D=64, num_t=64, num_s=256):
    qkv = x @ w_qkv.T + b_qkv           -> split to q, k, v per head
    template tokens 0:64   attend to tokens 0:64
    search   tokens 64:320 attend to all 320 tokens
    out = attn_out @ w_proj.T + b_proj

Data-parallel over batch across 8 NeuronCores (8 batches each). Host does all
layout transposes and dtype casts (bf16), plus two exact algebraic folds:
  - v-bias passes through softmax unchanged (rows sum to 1), so b_v is folded
    into an effective proj bias: b_proj_eff = b_proj + w_proj @ b_v.
  - b_proj_eff is pre-broadcast to [128, C] so the proj PSUM evacuation is a
    single tensor_tensor ADD (no rank-1 bias matmuls on the PE).
On-device dataflow per (batch, head):
  STk   = kT[d, kchunk].T @ qT[d, :]        (scores transposed, k on partitions,
                                             head pairs run row-group concurrent)
  PT    = exp(STk * 0.125)                  (ScalarE, PSUM -> SBUF, bf16)
  PV    = vaug[k, 65].T @ PT[k, q]          -> [65, 320]: rows 0:64 = attn outT,
                                               row 64 = colsums (ones column,
                                               written by a strided memset)
Softmax normalization is matmul-free: the 12 colsum rows are DMA-gathered into
a [128, 30] tile (one batched reciprocal at 8 cycles/element spread over 128
partitions), DMA-scattered back, then DMA-broadcast (free-dim stride-0 source)
into a [128, 1920] tile whose partition halves match the even/odd head packing
of the attention-out tiles; normalization is then 6 bf16 tensor_muls per batch.
Projection: out[t, co] = aT[c, t].T @ w_projT[c, co]; bias added during PSUM
evacuation via the pre-broadcast bias tile.
All matmul operands are bf16 (full PE rate at any moving size, FWL weight
loads); PSUM accumulation stays fp32. The v-projection's 64-token tail chunks
of the two batches in a pair are packed into one [128, 128] stationary (tails
DMA'd twice into a dedicated tile) so those matmuls use the full PE width.
The schedule is software-pipelined as in the baseline: pair p+1's dense qkv
matmuls are interleaved into pair p's attention phase to keep PE duty above
the HAM clock-gate threshold; weight DMAs are split (wqk in column halves) and
ordered so the first qkv matmul can start ~10us into the kernel.
"""

import sys

sys.path.insert(0, "/opt/trn_rl_repo")

import numpy as np
import ml_dtypes

B, N, C = 64, 320, 768
H, D = 12, 64
NT, NS = 64, 256
NCORES = 8
BC = B // NCORES  # batches per core
CCH = C // 128  # 6 contraction chunks
QK_TILES = (2 * C) // 128  # 12 co-tiles covering q and k sections
TCH = [(0, 128), (128, 128), (256, 64)]  # token chunks (t or k)
VW = H * 65  # 780: v width incl. ones columns
NPH = VW // 2  # 390: vnat free-dim half
PH = C // 2  # 384: proj free-dim half

_CACHE = {}


def _build():
    import concourse.bacc as bacc
    import concourse.mybir as mybir
    import concourse.tile as tile

    F32 = mybir.dt.float32
    BF16 = mybir.dt.bfloat16
    EXP = mybir.ActivationFunctionType.Exp

    nc = bacc.Bacc("TRN2")

    d_xt = nc.dram_tensor("xt", [BC, C, N], BF16, kind="ExternalInput")
    d_wqk = nc.dram_tensor("wqk", [C, 2 * C], BF16, kind="ExternalInput")
    d_wv = nc.dram_tensor("wv", [C, VW], BF16, kind="ExternalInput")
    d_wp = nc.dram_tensor("wp", [C, C], BF16, kind="ExternalInput")
    d_bqk = nc.dram_tensor("bqk", [128, QK_TILES], F32, kind="ExternalInput")
    d_bp = nc.dram_tensor("bp", [128, C], BF16, kind="ExternalInput")
    d_out = nc.dram_tensor("out", [BC, N, C], F32, kind="ExternalOutput")

    with tile.TileContext(nc) as tc:
        with (
            tc.tile_pool(name="const", bufs=1) as cp,
            tc.tile_pool(name="work", bufs=2) as wp,
            tc.tile_pool(name="psum", bufs=2, space="PSUM") as pp,
        ):
            # ---- resident weights; DMA order = first-use order ----
            bqk_sb = cp.tile([128, QK_TILES], F32, name="bqk", tag="bqk")
            nc.sync.dma_start(bqk_sb[:], d_bqk[:])

            def xt_dma(p):
                # one DMA per batch: [C, N] HBM -> [128, 6*N] SBUF (c-major
                # free dim), so the Sync engine issues 2 big DMAs instead of
                # 12 small ones (each dma_start costs ~650ns of issue time)
                xt_sb = {}
                bt = {}
                for b in (2 * p, 2 * p + 1):
                    t_xt = wp.tile(
                        [128, CCH * N], BF16, name=f"xt{b}", tag="xt", bufs=4
                    )
                    src = d_xt[b, :, :].rearrange("(c p) q -> p c q", p=128)
                    dst = t_xt[:, :].rearrange("p (c q) -> p c q", q=N)
                    nc.sync.dma_start(dst, src)
                    bt[b] = t_xt
                    for c in range(CCH):
                        xt_sb[(b, c)] = t_xt[:, c * N : (c + 1) * N]
                # tail tokens of both batches packed [b0 256:320 | b1 256:320]
                # per c-chunk, via 2 SBUF->SBUF DMAs
                t_tl = wp.tile([128, CCH * 128], BF16, name=f"xtl{p}", tag="xtl", bufs=2)
                for i, b in enumerate((2 * p, 2 * p + 1)):
                    src = bt[b][:, :].rearrange("p (c q) -> p c q", q=N)[:, :, 256:N]
                    dst = t_tl[:, :].rearrange("p (c i q) -> p c i q", i=2, q=64)[
                        :, :, i, :
                    ]
                    nc.sync.dma_start(dst, src)
                for c in range(CCH):
                    xt_sb[("tl", c)] = t_tl[:, c * 128 : (c + 1) * 128]
                return xt_sb

            pair_state = {0: {}}
            pair_state[0]["xt"] = xt_dma(0)

            wqk_sb = {}
            for hf in range(2):
                for c in range(CCH):
                    t_wqk = cp.tile(
                        [128, C], BF16, name=f"wqk{c}_{hf}", tag=f"wqk{c}_{hf}"
                    )
                    nc.sync.dma_start(
                        t_wqk[:], d_wqk[c * 128 : (c + 1) * 128, hf * C : (hf + 1) * C]
                    )
                    wqk_sb[(c, hf)] = t_wqk
            wv_sb = []
            wp_sb = []
            for c in range(CCH):
                t_wv = cp.tile([128, VW], BF16, name=f"wv{c}", tag=f"wv{c}")
                nc.sync.dma_start(t_wv[:], d_wv[c * 128 : (c + 1) * 128, :])
                wv_sb.append(t_wv)
            bp_sb = cp.tile([128, C], BF16, name="bp", tag="bp")
            nc.sync.dma_start(bp_sb[:], d_bp[:])
            for c in range(CCH):
                t_wp = cp.tile([128, C], BF16, name=f"wp{c}", tag=f"wp{c}")
                nc.sync.dma_start(t_wp[:], d_wp[c * 128 : (c + 1) * 128, :])
                wp_sb.append(t_wp)

            def attn_headpair(b, hp, qk_sb, vaug_sb, at_sb, sumsf):
                # head pair (2hp, 2hp+1): even head at partitions 0:64, odd
                # at 64:128 of the same qk tiles. The two score matmuls of a
                # chunk hit different PE row groups and run concurrently.
                qt = qk_sb[hp]
                kt = qk_sb[6 + hp]
                pt_sb = {0: [], 1: []}
                for ki, (k0, kl) in enumerate(TCH):
                    q0 = 0 if ki == 0 else 64
                    ps_pair = []
                    for par in range(2):
                        off = par * 64
                        ps = pp.tile(
                            [kl, N - q0],
                            F32,
                            name=f"pst{b}_{hp}_{par}_{ki}",
                            tag="pst",
                            bufs=3,
                        )
                        nc.tensor.matmul(
                            ps[:],
                            kt[off : off + 64, k0 : k0 + kl],
                            qt[off : off + 64, q0:N],
                            start=True,
                            stop=True,
                        )
                        ps_pair.append(ps)
                    for par in range(2):
                        t_pt = wp.tile(
                            [kl, N - q0],
                            BF16,
                            name=f"pt{b}_{hp}_{par}_{ki}",
                            tag="pt",
                            bufs=8,
                        )
                        nc.scalar.activation(t_pt[:], ps_pair[par][:], EXP, scale=0.125)
                        pt_sb[par].append(t_pt)
                for par in range(2):
                    h = 2 * hp + par
                    off = par * 64
                    pts = pt_sb[par]
                    # PV: rows 0:64 = attn outT (unnormalized), row 64 = colsums
                    po = pp.tile([65, N], F32, name=f"po{b}_{h}", tag="po", bufs=3)
                    hs = slice(h * 65, (h + 1) * 65)
                    nc.tensor.matmul(
                        po[:, 0:64],
                        vaug_sb[0][0:64, hs],
                        pts[0][0:64, 0:64],
                        start=True,
                        stop=False,
                    )
                    nc.tensor.matmul(
                        po[:, 64:N],
                        vaug_sb[0][:, hs],
                        pts[0][:, 64:N],
                        start=False,
                        stop=False,
                    )
                    nc.tensor.matmul(
                        po[:, 64:N], vaug_sb[1][:, hs], pts[1][:], start=False, stop=False
                    )
                    nc.tensor.matmul(
                        po[:, 64:N], vaug_sb[2][:, hs], pts[2][:], start=False, stop=True
                    )
                    # evacuate unnormalized rows + colsum row; frees the bank.
                    # sums layout: par-major [par*1920 + hp*320 + q] so the
                    # broadcast sources are contiguous per parity.
                    nc.any.tensor_copy(at_sb[hp][off : off + 64, :], po[0:64, :])
                    so = par * (6 * N) + hp * N
                    nc.any.tensor_copy(sumsf[0:1, so : so + N], po[64:65, :])

            def attn_chain(b, sumsf):
                # batched softmax reciprocals: spread the 12*N sums across all
                # 128 partitions (DVE reciprocal costs 8 cycles/elem serially
                # per partition), then scatter back flat and DMA-broadcast to
                # a [128, 1920] tile: partitions 0:64 = even heads, 64:128 =
                # odd heads, matching the attention-out tile packing.
                s128 = wp.tile([128, 30], BF16, name=f"s128_{b}", tag="s128", bufs=1)
                nc.sync.dma_start(
                    s128[:, :], sumsf[0:1, :].rearrange("o (p q) -> o p q", p=128)
                )
                rr = wp.tile([128, 30], BF16, name=f"rr{b}", tag="rr", bufs=1)
                with nc.allow_low_precision(reason="bf16 softmax reciprocal"):
                    nc.vector.reciprocal(rr[:], s128[:])
                rcpf = wp.tile([1, 12 * N], BF16, name=f"rcpf{b}", tag="rcpf", bufs=1)
                nc.sync.dma_start(
                    rcpf[0:1, :].rearrange("o (p q) -> o p q", p=128), rr[:, :]
                )
                bc = wp.tile([128, 6 * N], BF16, name=f"bc{b}", tag="bc", bufs=2)
                for par in range(2):
                    src = (
                        rcpf[0:1, par * 6 * N : (par + 1) * 6 * N]
                        .rearrange("o (b q) -> o b q", b=1)
                        .broadcast_to([1, 64, 6 * N])
                    )
                    nc.sync.dma_start(bc[par * 64 : (par + 1) * 64, :], src)
                return bc

            def norm_batch(b, at_sb, bc):
                for hp in range(6):
                    with nc.allow_low_precision(reason="bf16 attn normalize"):
                        nc.vector.tensor_mul(
                            at_sb[hp][:, :],
                            at_sb[hp][:, :],
                            bc[:, hp * N : (hp + 1) * N],
                        )

            def proj_unit(b, ti, at_sb):
                t0, tl = TCH[ti]
                t_o = wp.tile([tl, C], BF16, name=f"outp{b}_{ti}", tag="outp", bufs=3)
                ps_h = [
                    pp.tile([tl, PH], F32, name=f"psp{b}_{ti}_{nh}", tag="pmm", bufs=2)
                    for nh in range(2)
                ]
                for c in range(CCH):
                    for nh in range(2):
                        nc.tensor.matmul(
                            ps_h[nh][:],
                            at_sb[c][:, t0 : t0 + tl],
                            wp_sb[c][:, nh * PH : (nh + 1) * PH],
                            start=(c == 0),
                            stop=(c == CCH - 1),
                        )
                for nh in range(2):
                    with nc.allow_low_precision(reason="bf16 out staging"):
                        nc.vector.tensor_add(
                            t_o[:, nh * PH : (nh + 1) * PH],
                            ps_h[nh][:],
                            bp_sb[0:tl, nh * PH : (nh + 1) * PH],
                        )
                # gpsimd-initiated DMA widens bf16 -> fp32 on the way out
                nc.gpsimd.dma_start(d_out[b, t0 : t0 + tl, :], t_o[:])

            def _vnat_evac(t_v, rows, ps_h):
                for nh in range(2):
                    nc.any.tensor_copy(
                        t_v[:, nh * NPH : (nh + 1) * NPH], ps_h[nh][rows, :]
                    )
                ones_ap = t_v[:, :].rearrange("p (h c) -> p h c", c=65)[:, :, 64:65]
                nc.gpsimd.memset(ones_ap, 1.0)

            def vnat_unit(b, ti, xt_sb):
                # head chunks 0/1 of one batch: [128, VW] stationary
                t0, tl = TCH[ti]
                t_v = wp.tile([tl, VW], BF16, name=f"vaug{b}_{ti}", tag="vaug", bufs=8)
                ps_h = [
                    pp.tile([tl, NPH], F32, name=f"psv{b}_{ti}_{nh}", tag="pmm", bufs=2)
                    for nh in range(2)
                ]
                for c in range(CCH):
                    for nh in range(2):
                        nc.tensor.matmul(
                            ps_h[nh][:],
                            xt_sb[(b, c)][:, t0 : t0 + tl],
                            wv_sb[c][:, nh * NPH : (nh + 1) * NPH],
                            start=(c == 0),
                            stop=(c == CCH - 1),
                        )
                _vnat_evac(t_v, slice(0, tl), ps_h)
                return t_v

            def vnat_tail(p, xt_sb):
                # both batches' 64-token tails in one [128, 128] stationary
                t_v0 = wp.tile([64, VW], BF16, name=f"vaugt{2*p}", tag="vaug", bufs=8)
                t_v1 = wp.tile([64, VW], BF16, name=f"vaugt{2*p+1}", tag="vaug", bufs=8)
                ps_h = [
                    pp.tile([128, NPH], F32, name=f"psvt{p}_{nh}", tag="pmm", bufs=2)
                    for nh in range(2)
                ]
                for c in range(CCH):
                    for nh in range(2):
                        nc.tensor.matmul(
                            ps_h[nh][:],
                            xt_sb[("tl", c)],
                            wv_sb[c][:, nh * NPH : (nh + 1) * NPH],
                            start=(c == 0),
                            stop=(c == CCH - 1),
                        )
                _vnat_evac(t_v0, slice(0, 64), ps_h)
                _vnat_evac(t_v1, slice(64, 128), ps_h)
                return t_v0, t_v1

            def qkv_unit(p, j, xt_sb, qk_sb):
                # qkT projection for one co-tile, batch-paired so the weight
                # tile is stationary across two consecutive matmuls
                bpair = (2 * p, 2 * p + 1)
                hf, jc = j // 6, j % 6
                ps_b = {
                    b: pp.tile([128, N], F32, name=f"psqk{b}_{j}", tag="pmm", bufs=2)
                    for b in bpair
                }
                for c in range(CCH):
                    for b in bpair:
                        nc.tensor.matmul(
                            ps_b[b][:],
                            wqk_sb[(c, hf)][:, jc * 128 : (jc + 1) * 128],
                            xt_sb[(b, c)],
                            start=(c == 0),
                            stop=(c == CCH - 1),
                        )
                for b in bpair:
                    t_qk = wp.tile([128, N], BF16, name=f"qk{b}_{j}", tag="qkt", bufs=42)
                    with nc.allow_low_precision(reason="bf16 q/k for scores"):
                        nc.vector.tensor_scalar_add(
                            t_qk[:], ps_b[b][:], bqk_sb[:, j : j + 1]
                        )
                    qk_sb[b].append(t_qk)

            def emit_pair(p, qk_sb, filler):
                """attention + normalize + projection for pair p, with
                filler() hooks where the driver injects the next pair's dense
                qkv/v-projection matmuls to keep the PE array duty above the
                HAM clock-gate threshold (the PE runs at 1.2 GHz instead of
                2.4 when its duty drops for a ~3.4us window)."""
                b0, b1 = 2 * p, 2 * p + 1
                xt_sb = pair_state[p]["xt"]
                V = pair_state[p]["vaug"]
                vaug0 = [V[0], V[1], V["t0"]]
                at0 = [
                    wp.tile([128, N], BF16, name=f"at{b0}_{j}", tag="at", bufs=12)
                    for j in range(CCH)
                ]
                sumsf0 = wp.tile(
                    [1, H * N], BF16, name=f"sumsf{b0}", tag="sumsf", bufs=1
                )
                vaug1 = [None, None, V["t1"]]
                for hp in range(H // 2):
                    attn_headpair(b0, hp, qk_sb[b0], vaug0, at0, sumsf0)
                    if hp == 0:
                        vaug1[0] = vnat_unit(b1, 0, xt_sb)
                    elif hp == 2:
                        vaug1[1] = vnat_unit(b1, 1, xt_sb)
                    else:
                        filler()
                bc0 = attn_chain(b0, sumsf0)
                at1 = [
                    wp.tile([128, N], BF16, name=f"at{b1}_{j}", tag="at", bufs=12)
                    for j in range(CCH)
                ]
                sumsf1 = wp.tile(
                    [1, H * N], BF16, name=f"sumsf{b1}", tag="sumsf", bufs=1
                )
                # b0's normalize+projection folds into b1's attention so the
                # PE never head-of-line blocks on b0's reciprocal DMA chain:
                # by the time the proj matmuls reach the PE FIFO, three of
                # b1's head pairs are queued ahead of them.
                for hp in range(H // 2):
                    attn_headpair(b1, hp, qk_sb[b1], vaug1, at1, sumsf1)
                    if hp == 3:
                        norm_batch(b0, at0, bc0)
                    elif hp == 4:
                        proj_unit(b0, 0, at0)
                    elif hp == 5:
                        proj_unit(b0, 1, at0)
                    else:
                        filler()
                filler()
                proj_unit(b0, 2, at0)
                filler()
                bc1 = attn_chain(b1, sumsf1)
                filler()
                norm_batch(b1, at1, bc1)
                filler()
                proj_unit(b1, 0, at1)
                filler()
                proj_unit(b1, 1, at1)
                filler()
                proj_unit(b1, 2, at1)

            def emit_vnat_pair(p):
                xt_sb = pair_state[p]["xt"]
                vt0, vt1 = vnat_tail(p, xt_sb)
                V = {"t0": vt0, "t1": vt1}
                V[0] = vnat_unit(2 * p, 0, xt_sb)
                V[1] = vnat_unit(2 * p, 1, xt_sb)
                pair_state[p]["vaug"] = V

            # ---- software-pipelined driver: pair p+1's dense qkv and
            # v-projection matmuls are emitted interleaved into pair p's
            # attention/projection phase, paced evenly over the filler sites
            # so the PE never sees a long matmul-free window ----
            NP = BC // 2
            for p in range(NP):
                pair_state.setdefault(p, {})
                pair_state[p]["qk"] = {2 * p: [], 2 * p + 1: []}
            for j in range(QK_TILES):
                qkv_unit(0, j, pair_state[0]["xt"], pair_state[0]["qk"])
            emit_vnat_pair(0)
            N_SITES = 13  # filler() call sites per non-last emit_pair
            for p in range(NP):
                if p + 1 < NP:
                    pair_state[p + 1]["xt"] = xt_dma(p + 1)
                    nxt = pair_state[p + 1]
                    units = [
                        (lambda j=j, pn=p + 1, nxt=nxt: qkv_unit(
                            pn, j, nxt["xt"], nxt["qk"]
                        ))
                        for j in range(QK_TILES)
                    ]
                    units.append(lambda pn=p + 1: emit_vnat_pair(pn))
                    st = {"site": 0, "done": 0}

                    def filler(units=units, st=st):
                        st["site"] += 1
                        tgt = st["site"] * len(units) // N_SITES
                        while st["done"] < min(tgt, len(units)):
                            units[st["done"]]()
                            st["done"] += 1
                else:
                    pair_state[p]["last"] = True

                    def filler():
                        pass
                emit_pair(p, pair_state[p]["qk"], filler)

    nc.compile()
    return nc


def _get_nc():
    if "nc" not in _CACHE:
        _CACHE["nc"] = _build()
    return _CACHE["nc"]


def _host_prep(x, w_qkv, b_qkv, w_proj, b_proj):
    x = np.asarray(x, dtype=np.float32)
    w_qkv = np.asarray(w_qkv, dtype=np.float32)
    b_qkv = np.asarray(b_qkv, dtype=np.float32)
    w_proj = np.asarray(w_proj, dtype=np.float32)
    b_proj = np.asarray(b_proj, dtype=np.float32)
    bf16 = ml_dtypes.bfloat16

    xt = np.ascontiguousarray(x.transpose(0, 2, 1)).astype(bf16)  # [B, C, N]
    wqk = np.ascontiguousarray(w_qkv[: 2 * C].T).astype(bf16)  # [C, 2C]
    wv_nat = w_qkv[2 * C :]  # [C(hd), C(c)]
    wv = np.zeros((C, VW), dtype=np.float32)
    for h in range(H):
        wv[:, h * 65 : h * 65 + 64] = wv_nat[h * 64 : (h + 1) * 64].T
    wv = wv.astype(bf16)
    bqk = np.ascontiguousarray(b_qkv[: 2 * C].reshape(QK_TILES, 128).T)  # [128, 12]
    wpr = np.ascontiguousarray(w_proj.T).astype(bf16)  # [C, C]
    # v-bias passes through softmax (rows sum to 1): fold into proj bias,
    # then pre-broadcast to [128, C] for the tensor_tensor bias add.
    bp_eff = b_proj + w_proj @ b_qkv[2 * C :]
    bp = np.broadcast_to(bp_eff.reshape(1, C), (128, C)).astype(bf16)
    bp = np.ascontiguousarray(bp)
    return xt, wqk, wv, wpr, bqk, bp


def _run(x, w_qkv, b_qkv, w_proj, b_proj, trace=False, trace_cores=None):
    from concourse.bass_utils import run_bass_kernel_spmd

    xt, wqk, wv, wpr, bqk, bp = _host_prep(x, w_qkv, b_qkv, w_proj, b_proj)
    nc = _get_nc()
    in_maps = []
    for i in range(NCORES):
        in_maps.append(
            {
                "xt": xt[i * BC : (i + 1) * BC],
                "wqk": wqk,
                "wv": wv,
                "wp": wpr,
                "bqk": bqk,
                "bp": bp,
            }
        )
    kwargs = {}
    if trace:
        kwargs = {"trace": True, "trace_cores": trace_cores or [0]}
    res = run_bass_kernel_spmd(nc, in_maps, core_ids=list(range(NCORES)), **kwargs)
    out = np.concatenate([res.results[i]["out"] for i in range(NCORES)], axis=0)
    return out.astype(np.float32), res


def kernel(x, w_qkv, b_qkv, w_proj, b_proj, num_t, num_s):
    assert int(num_t) == NT and int(num_s) == NS
    out, _ = _run(x, w_qkv, b_qkv, w_proj, b_proj)
    return out


# revision 16
# speedup vs baseline: 1.0116x; 1.0116x over previous
"""Sparse attention (template/search) Trainium2 Bass kernel.

Reference computation (B=64, N=320, C=768, H=12, D=64, num_t=64, num_s=256):
    qkv = x @ w_qkv.T + b_qkv           -> split to q, k, v per head
    template tokens 0:64   attend to tokens 0:64
    search   tokens 64:320 attend to all 320 tokens
    out = attn_out @ w_proj.T + b_proj

Data-parallel over batch across 8 NeuronCores (8 batches each). Host does all
layout transposes and dtype casts (bf16), plus two exact algebraic folds:
  - v-bias passes through softmax unchanged (rows sum to 1), so b_v is folded
    into an effective proj bias: b_proj_eff = b_proj + w_proj @ b_v.
  - b_proj_eff is pre-broadcast to [128, C] so the proj PSUM evacuation is a
    single tensor_tensor ADD (no rank-1 bias matmuls on the PE).
On-device dataflow per (batch, head):
  STk   = kT[d, kchunk].T @ qT[d, :]        (scores transposed, k on partitions,
                                             head pairs run row-group concurrent)
  PT    = exp(STk * 0.125)                  (ScalarE, PSUM -> SBUF, bf16)
  PV    = vaug[k, 65].T @ PT[k, q]          -> [65, 320]: rows 0:64 = attn outT,
                                               row 64 = colsums (ones column,
                                               written by a strided memset)
Softmax normalization is matmul-free: the 12 colsum rows are DMA-gathered into
a [128, 30] tile (one batched reciprocal at 8 cycles/element spread over 128
partitions), DMA-scattered back, then DMA-broadcast (free-dim stride-0 source)
into a [128, 1920] tile whose partition halves match the even/odd head packing
of the attention-out tiles; normalization is then 6 bf16 tensor_muls per batch.
Projection: out[t, co] = aT[c, t].T @ w_projT[c, co]; bias added during PSUM
evacuation via the pre-broadcast bias tile.
All matmul operands are bf16 (full PE rate at any moving size, FWL weight
loads); PSUM accumulation stays fp32. The v-projection's 64-token tail chunks
of the two batches in a pair are packed into one [128, 128] stationary (tails
DMA'd twice into a dedicated tile) so those matmuls use the full PE width.
The schedule is software-pipelined as in the baseline: pair p+1's dense qkv
matmuls are interleaved into pair p's attention phase to keep PE duty above
the HAM clock-gate threshold; weight DMAs are split (wqk in column halves) and
ordered so the first qkv matmul can start ~10us into the kernel.
"""

import sys

sys.path.insert(0, "/opt/trn_rl_repo")

import numpy as np
import ml_dtypes

B, N, C = 64, 320, 768
H, D = 12, 64
NT, NS = 64, 256
NCORES = 8
BC = B // NCORES  # batches per core
CCH = C // 128  # 6 contraction chunks
QK_TILES = (2 * C) // 128  # 12 co-tiles covering q and k sections
TCH = [(0, 128), (128, 128), (256, 64)]  # token chunks (t or k)
VW = H * 65  # 780: v width incl. ones columns
NPH = VW // 2  # 390: vnat free-dim half
PH = C // 2  # 384: proj free-dim half

_CACHE = {}


def _build():
    import concourse.bacc as bacc
    import concourse.mybir as mybir
    import concourse.tile as tile

    F32 = mybir.dt.float32
    BF16 = mybir.dt.bfloat16
    EXP = mybir.ActivationFunctionType.Exp

    nc = bacc.Bacc("TRN2")

    d_xt = nc.dram_tensor("xt", [BC, C, N], BF16, kind="ExternalInput")
    d_wqk = nc.dram_tensor("wqk", [C, 2 * C], BF16, kind="ExternalInput")
    d_wv = nc.dram_tensor("wv", [C, VW], BF16, kind="ExternalInput")
    d_wp = nc.dram_tensor("wp", [C, C], BF16, kind="ExternalInput")
    d_bqk = nc.dram_tensor("bqk", [128, QK_TILES], F32, kind="ExternalInput")
    d_bp = nc.dram_tensor("bp", [128, C], BF16, kind="ExternalInput")
    d_out = nc.dram_tensor("out", [BC, N, C], F32, kind="ExternalOutput")

    with tile.TileContext(nc) as tc:
        with (
            tc.tile_pool(name="const", bufs=1) as cp,
            tc.tile_pool(name="work", bufs=2) as wp,
            tc.tile_pool(name="psum", bufs=2, space="PSUM") as pp,
        ):
            # ---- resident weights; DMA order = first-use order ----
            bqk_sb = cp.tile([128, QK_TILES], F32, name="bqk", tag="bqk")
            nc.sync.dma_start(bqk_sb[:], d_bqk[:])

            def xt_dma(p):
                # one DMA per batch: [C, N] HBM -> [128, 6*N] SBUF (c-major
                # free dim), so the Sync engine issues 2 big DMAs instead of
                # 12 small ones (each dma_start costs ~650ns of issue time)
                xt_sb = {}
                bt = {}
                for b in (2 * p, 2 * p + 1):
                    t_xt = wp.tile(
                        [128, CCH * N], BF16, name=f"xt{b}", tag="xt", bufs=4
                    )
                    src = d_xt[b, :, :].rearrange("(c p) q -> p c q", p=128)
                    dst = t_xt[:, :].rearrange("p (c q) -> p c q", q=N)
                    nc.sync.dma_start(dst, src)
                    bt[b] = t_xt
                    for c in range(CCH):
                        xt_sb[(b, c)] = t_xt[:, c * N : (c + 1) * N]
                # tail tokens of both batches packed [b0 256:320 | b1 256:320]
                # per c-chunk, via 2 SBUF->SBUF DMAs
                t_tl = wp.tile([128, CCH * 128], BF16, name=f"xtl{p}", tag="xtl", bufs=2)
                for i, b in enumerate((2 * p, 2 * p + 1)):
                    src = bt[b][:, :].rearrange("p (c q) -> p c q", q=N)[:, :, 256:N]
                    dst = t_tl[:, :].rearrange("p (c i q) -> p c i q", i=2, q=64)[
                        :, :, i, :
                    ]
                    nc.sync.dma_start(dst, src)
                for c in range(CCH):
                    xt_sb[("tl", c)] = t_tl[:, c * 128 : (c + 1) * 128]
                return xt_sb

            pair_state = {0: {}}
            pair_state[0]["xt"] = xt_dma(0)

            wqk_sb = {}
            for hf in range(2):
                for c in range(CCH):
                    t_wqk = cp.tile(
                        [128, C], BF16, name=f"wqk{c}_{hf}", tag=f"wqk{c}_{hf}"
                    )
                    nc.sync.dma_start(
                        t_wqk[:], d_wqk[c * 128 : (c + 1) * 128, hf * C : (hf + 1) * C]
                    )
                    wqk_sb[(c, hf)] = t_wqk
            wv_sb = []
            wp_sb = []
            for c in range(CCH):
                t_wv = cp.tile([128, VW], BF16, name=f"wv{c}", tag=f"wv{c}")
                nc.sync.dma_start(t_wv[:], d_wv[c * 128 : (c + 1) * 128, :])
                wv_sb.append(t_wv)
            bp_sb = cp.tile([128, C], BF16, name="bp", tag="bp")
            nc.sync.dma_start(bp_sb[:], d_bp[:])
            for c in range(CCH):
                t_wp = cp.tile([128, C], BF16, name=f"wp{c}", tag=f"wp{c}")
                nc.sync.dma_start(t_wp[:], d_wp[c * 128 : (c + 1) * 128, :])
                wp_sb.append(t_wp)

            def attn_headpair(b, hp, qk_sb, vaug_sb, at_sb, sumsf):
                # head pair (2hp, 2hp+1): even head at partitions 0:64, odd
                # at 64:128 of the same qk tiles. The two score matmuls of a
                # chunk hit different PE row groups and run concurrently.
                qt = qk_sb[hp]
                kt = qk_sb[6 + hp]
                pt_sb = {0: [], 1: []}
                for ki, (k0, kl) in enumerate(TCH):
                    q0 = 0 if ki == 0 else 64
                    ps_pair = []
                    for par in range(2):
                        off = par * 64
                        ps = pp.tile(
                            [kl, N - q0],
                            F32,
                            name=f"pst{b}_{hp}_{par}_{ki}",
                            tag="pst",
                            bufs=3,
                        )
                        nc.tensor.matmul(
                            ps[:],
                            kt[off : off + 64, k0 : k0 + kl],
                            qt[off : off + 64, q0:N],
                            start=True,
                            stop=True,
                        )
                        ps_pair.append(ps)
                    for par in range(2):
                        t_pt = wp.tile(
                            [kl, N - q0],
                            BF16,
                            name=f"pt{b}_{hp}_{par}_{ki}",
                            tag="pt",
                            bufs=8,
                        )
                        nc.scalar.activation(t_pt[:], ps_pair[par][:], EXP, scale=0.125)
                        pt_sb[par].append(t_pt)
                for par in range(2):
                    h = 2 * hp + par
                    off = par * 64
                    pts = pt_sb[par]
                    # PV: rows 0:64 = attn outT (unnormalized), row 64 = colsums
                    po = pp.tile([65, N], F32, name=f"po{b}_{h}", tag="po", bufs=3)
                    hs = slice(h * 65, (h + 1) * 65)
                    nc.tensor.matmul(
                        po[:, 0:64],
                        vaug_sb[0][0:64, hs],
                        pts[0][0:64, 0:64],
                        start=True,
                        stop=False,
                    )
                    nc.tensor.matmul(
                        po[:, 64:N],
                        vaug_sb[0][:, hs],
                        pts[0][:, 64:N],
                        start=False,
                        stop=False,
                    )
                    nc.tensor.matmul(
                        po[:, 64:N], vaug_sb[1][:, hs], pts[1][:], start=False, stop=False
                    )
                    nc.tensor.matmul(
                        po[:, 64:N], vaug_sb[2][:, hs], pts[2][:], start=False, stop=True
                    )
                    # evacuate unnormalized rows + colsum row; frees the bank.
                    # sums go to the half-batch tile (head pairs 0:3 / 3:6),
                    # par-major [par*960 + (hp%3)*320 + q] so the broadcast
                    # sources are contiguous per parity.
                    nc.any.tensor_copy(at_sb[hp][off : off + 64, :], po[0:64, :])
                    so = par * (3 * N) + (hp % 3) * N
                    sf = sumsf[0] if hp < 3 else sumsf[1]
                    nc.any.tensor_copy(sf[0:1, so : so + N], po[64:65, :])

            def attn_chain(b, sf, half):
                # batched softmax reciprocals for one half-batch (3 head
                # pairs): gather the 6*N sums across 64 partitions (DVE
                # reciprocal costs 8 cycles/elem serially per partition),
                # scatter back flat, then DMA-broadcast (free-dim stride-0
                # source) to a [128, 3*N] tile whose partition halves match
                # the even/odd head packing of the attention-out tiles.
                # Split in halves so normalization can start 3 head pairs
                # earlier and the chain latency hides behind attention.
                HN = 3 * N
                s64 = wp.tile([64, 30], BF16, name=f"s64_{b}_{half}", tag="s64", bufs=2)
                nc.sync.dma_start(
                    s64[:, :], sf[0:1, :].rearrange("o (p q) -> o p q", p=64)
                )
                rr = wp.tile([64, 30], BF16, name=f"rr{b}_{half}", tag="rr", bufs=2)
                with nc.allow_low_precision(reason="bf16 softmax reciprocal"):
                    nc.vector.reciprocal(rr[:], s64[:])
                rcpf = wp.tile([1, 2 * HN], BF16, name=f"rcpf{b}_{half}", tag="rcpf", bufs=2)
                nc.sync.dma_start(
                    rcpf[0:1, :].rearrange("o (p q) -> o p q", p=64), rr[:, :]
                )
                bc = wp.tile([128, HN], BF16, name=f"bc{b}_{half}", tag="bc", bufs=4)
                for par in range(2):
                    src = (
                        rcpf[0:1, par * HN : (par + 1) * HN]
                        .rearrange("o (b q) -> o b q", b=1)
                        .broadcast_to([1, 64, HN])
                    )
                    nc.sync.dma_start(bc[par * 64 : (par + 1) * 64, :], src)
                return bc

            def norm_half(b, at_sb, bc, half):
                for hp in range(3 * half, 3 * half + 3):
                    with nc.allow_low_precision(reason="bf16 attn normalize"):
                        nc.vector.tensor_mul(
                            at_sb[hp][:, :],
                            at_sb[hp][:, :],
                            bc[:, (hp % 3) * N : (hp % 3 + 1) * N],
                        )

            def proj_unit(b, ti, at_sb):
                t0, tl = TCH[ti]
                t_o = wp.tile([tl, C], BF16, name=f"outp{b}_{ti}", tag="outp", bufs=3)
                ps_h = [
                    pp.tile([tl, PH], F32, name=f"psp{b}_{ti}_{nh}", tag="pmm", bufs=2)
                    for nh in range(2)
                ]
                for c in range(CCH):
                    for nh in range(2):
                        nc.tensor.matmul(
                            ps_h[nh][:],
                            at_sb[c][:, t0 : t0 + tl],
                            wp_sb[c][:, nh * PH : (nh + 1) * PH],
                            start=(c == 0),
                            stop=(c == CCH - 1),
                        )
                for nh in range(2):
                    with nc.allow_low_precision(reason="bf16 out staging"):
                        nc.vector.tensor_add(
                            t_o[:, nh * PH : (nh + 1) * PH],
                            ps_h[nh][:],
                            bp_sb[0:tl, nh * PH : (nh + 1) * PH],
                        )
                # gpsimd-initiated DMA widens bf16 -> fp32 on the way out
                nc.gpsimd.dma_start(d_out[b, t0 : t0 + tl, :], t_o[:])

            def _vnat_evac(t_v, rows, ps_h):
                for nh in range(2):
                    nc.any.tensor_copy(
                        t_v[:, nh * NPH : (nh + 1) * NPH], ps_h[nh][rows, :]
                    )
                ones_ap = t_v[:, :].rearrange("p (h c) -> p h c", c=65)[:, :, 64:65]
                nc.gpsimd.memset(ones_ap, 1.0)

            def vnat_unit(b, ti, xt_sb):
                # head chunks 0/1 of one batch: [128, VW] stationary
                t0, tl = TCH[ti]
                t_v = wp.tile([tl, VW], BF16, name=f"vaug{b}_{ti}", tag="vaug", bufs=8)
                ps_h = [
                    pp.tile([tl, NPH], F32, name=f"psv{b}_{ti}_{nh}", tag="pmm", bufs=2)
                    for nh in range(2)
                ]
                for c in range(CCH):
                    for nh in range(2):
                        nc.tensor.matmul(
                            ps_h[nh][:],
                            xt_sb[(b, c)][:, t0 : t0 + tl],
                            wv_sb[c][:, nh * NPH : (nh + 1) * NPH],
                            start=(c == 0),
                            stop=(c == CCH - 1),
                        )
                _vnat_evac(t_v, slice(0, tl), ps_h)
                return t_v

            def vnat_tail(p, xt_sb):
                # both batches' 64-token tails in one [128, 128] stationary
                t_v0 = wp.tile([64, VW], BF16, name=f"vaugt{2*p}", tag="vaug", bufs=8)
                t_v1 = wp.tile([64, VW], BF16, name=f"vaugt{2*p+1}", tag="vaug", bufs=8)
                ps_h = [
                    pp.tile([128, NPH], F32, name=f"psvt{p}_{nh}", tag="pmm", bufs=2)
                    for nh in range(2)
                ]
                for c in range(CCH):
                    for nh in range(2):
                        nc.tensor.matmul(
                            ps_h[nh][:],
                            xt_sb[("tl", c)],
                            wv_sb[c][:, nh * NPH : (nh + 1) * NPH],
                            start=(c == 0),
                            stop=(c == CCH - 1),
                        )
                _vnat_evac(t_v0, slice(0, 64), ps_h)
                _vnat_evac(t_v1, slice(64, 128), ps_h)
                return t_v0, t_v1

            def qkv_unit(p, j, xt_sb, qk_sb):
                # qkT projection for one co-tile, batch-paired so the weight
                # tile is stationary across two consecutive matmuls
                bpair = (2 * p, 2 * p + 1)
                hf, jc = j // 6, j % 6
                ps_b = {
                    b: pp.tile([128, N], F32, name=f"psqk{b}_{j}", tag="pmm", bufs=2)
                    for b in bpair
                }
                for c in range(CCH):
                    for b in bpair:
                        nc.tensor.matmul(
                            ps_b[b][:],
                            wqk_sb[(c, hf)][:, jc * 128 : (jc + 1) * 128],
                            xt_sb[(b, c)],
                            start=(c == 0),
                            stop=(c == CCH - 1),
                        )
                for b in bpair:
                    t_qk = wp.tile([128, N], BF16, name=f"qk{b}_{j}", tag="qkt", bufs=42)
                    with nc.allow_low_precision(reason="bf16 q/k for scores"):
                        nc.vector.tensor_scalar_add(
                            t_qk[:], ps_b[b][:], bqk_sb[:, j : j + 1]
                        )
                    qk_sb[b].append(t_qk)

            def emit_pair(p, qk_sb, filler, vnat_hook):
                """attention + normalize + projection for pair p. filler()
                injects one of the next pair's dense qkv units; vnat_hook()
                injects the next pair's v-projection units into the shadow of
                b1's reciprocal DMA chain. b0's normalize+projection folds
                into b1's attention so the PE never head-of-line blocks on
                b0's chain. All to keep the PE duty above the HAM clock-gate
                threshold (the PE runs at 1.2 GHz instead of 2.4 when its
                duty drops for a ~3.4us window)."""
                b0, b1 = 2 * p, 2 * p + 1
                xt_sb = pair_state[p]["xt"]
                V = pair_state[p]["vaug"]
                vaug0 = [V[0], V[1], V["t0"]]
                at0 = [
                    wp.tile([128, N], BF16, name=f"at{b0}_{j}", tag="at", bufs=12)
                    for j in range(CCH)
                ]
                sumsf0 = [
                    wp.tile([1, H * N // 2], BF16, name=f"sumsf{b0}_{h}", tag="sumsf", bufs=4)
                    for h in range(2)
                ]
                vaug1 = [None, None, V["t1"]]
                bc0 = [None, None]
                for hp in range(H // 2):
                    attn_headpair(b0, hp, qk_sb[b0], vaug0, at0, sumsf0)
                    if hp == 0:
                        vaug1[0] = vnat_unit(b1, 0, xt_sb)
                    elif hp == 2:
                        bc0[0] = attn_chain(b0, sumsf0[0], 0)
                        vaug1[1] = vnat_unit(b1, 1, xt_sb)
                    else:
                        filler()
                bc0[1] = attn_chain(b0, sumsf0[1], 1)
                at1 = [
                    wp.tile([128, N], BF16, name=f"at{b1}_{j}", tag="at", bufs=12)
                    for j in range(CCH)
                ]
                sumsf1 = [
                    wp.tile([1, H * N // 2], BF16, name=f"sumsf{b1}_{h}", tag="sumsf", bufs=4)
                    for h in range(2)
                ]
                bc1 = [None, None]
                for hp in range(H // 2):
                    attn_headpair(b1, hp, qk_sb[b1], vaug1, at1, sumsf1)
                    if hp == 0:
                        filler()
                    elif hp == 1:
                        norm_half(b0, at0, bc0[0], 0)
                    elif hp == 2:
                        bc1[0] = attn_chain(b1, sumsf1[0], 0)
                        filler()
                    elif hp == 3:
                        norm_half(b0, at0, bc0[1], 1)
                    elif hp == 4:
                        proj_unit(b0, 0, at0)
                    elif hp == 5:
                        proj_unit(b0, 1, at0)
                filler()
                proj_unit(b0, 2, at0)
                bc1[1] = attn_chain(b1, sumsf1[1], 1)
                vnat_hook()
                norm_half(b1, at1, bc1[0], 0)
                filler()
                norm_half(b1, at1, bc1[1], 1)
                filler()
                proj_unit(b1, 0, at1)
                filler()
                proj_unit(b1, 1, at1)
                filler()
                proj_unit(b1, 2, at1)
                filler()

            def emit_vnat_pair(p):
                xt_sb = pair_state[p]["xt"]
                vt0, vt1 = vnat_tail(p, xt_sb)
                V = {"t0": vt0, "t1": vt1}
                V[0] = vnat_unit(2 * p, 0, xt_sb)
                V[1] = vnat_unit(2 * p, 1, xt_sb)
                pair_state[p]["vaug"] = V

            # ---- software-pipelined driver: pair p+1's dense qkv and
            # v-projection matmuls are emitted interleaved into pair p's
            # attention/projection phase, paced evenly over the filler sites
            # so the PE never sees a long matmul-free window ----
            NP = BC // 2
            for p in range(NP):
                pair_state.setdefault(p, {})
                pair_state[p]["qk"] = {2 * p: [], 2 * p + 1: []}
            for j in range(QK_TILES):
                qkv_unit(0, j, pair_state[0]["xt"], pair_state[0]["qk"])
            emit_vnat_pair(0)
            N_SITES = 12  # filler() call sites per emit_pair
            for p in range(NP):
                if p + 1 < NP:
                    pair_state[p + 1]["xt"] = xt_dma(p + 1)
                    nxt = pair_state[p + 1]
                    units = [
                        (lambda j=j, pn=p + 1, nxt=nxt: qkv_unit(
                            pn, j, nxt["xt"], nxt["qk"]
                        ))
                        for j in range(QK_TILES)
                    ]
                    st = {"site": 0, "done": 0}

                    def filler(units=units, st=st):
                        st["site"] += 1
                        tgt = st["site"] * len(units) // N_SITES
                        while st["done"] < min(tgt, len(units)):
                            units[st["done"]]()
                            st["done"] += 1

                    def vnat_hook(pn=p + 1):
                        emit_vnat_pair(pn)
                else:

                    def filler():
                        pass

                    def vnat_hook():
                        pass
                emit_pair(p, pair_state[p]["qk"], filler, vnat_hook)

    nc.compile()
    return nc


def _get_nc():
    if "nc" not in _CACHE:
        _CACHE["nc"] = _build()
    return _CACHE["nc"]


def _host_prep(x, w_qkv, b_qkv, w_proj, b_proj):
    x = np.asarray(x, dtype=np.float32)
    w_qkv = np.asarray(w_qkv, dtype=np.float32)
    b_qkv = np.asarray(b_qkv, dtype=np.float32)
    w_proj = np.asarray(w_proj, dtype=np.float32)
    b_proj = np.asarray(b_proj, dtype=np.float32)
    bf16 = ml_dtypes.bfloat16

    xt = np.ascontiguousarray(x.transpose(0, 2, 1)).astype(bf16)  # [B, C, N]
    wqk = np.ascontiguousarray(w_qkv[: 2 * C].T).astype(bf16)  # [C, 2C]
    wv_nat = w_qkv[2 * C :]  # [C(hd), C(c)]
    wv = np.zeros((C, VW), dtype=np.float32)
    for h in range(H):
        wv[:, h * 65 : h * 65 + 64] = wv_nat[h * 64 : (h + 1) * 64].T
    wv = wv.astype(bf16)
    bqk = np.ascontiguousarray(b_qkv[: 2 * C].reshape(QK_TILES, 128).T)  # [128, 12]
    wpr = np.ascontiguousarray(w_proj.T).astype(bf16)  # [C, C]
    # v-bias passes through softmax (rows sum to 1): fold into proj bias,
    # then pre-broadcast to [128, C] for the tensor_tensor bias add.
    bp_eff = b_proj + w_proj @ b_qkv[2 * C :]
    bp = np.broadcast_to(bp_eff.reshape(1, C), (128, C)).astype(bf16)
    bp = np.ascontiguousarray(bp)
    return xt, wqk, wv, wpr, bqk, bp


def _run(x, w_qkv, b_qkv, w_proj, b_proj, trace=False, trace_cores=None):
    from concourse.bass_utils import run_bass_kernel_spmd

    xt, wqk, wv, wpr, bqk, bp = _host_prep(x, w_qkv, b_qkv, w_proj, b_proj)
    nc = _get_nc()
    in_maps = []
    for i in range(NCORES):
        in_maps.append(
            {
                "xt": xt[i * BC : (i + 1) * BC],
                "wqk": wqk,
                "wv": wv,
                "wp": wpr,
                "bqk": bqk,
                "bp": bp,
            }
        )
    kwargs = {}
    if trace:
        kwargs = {"trace": True, "trace_cores": trace_cores or [0]}
    res = run_bass_kernel_spmd(nc, in_maps, core_ids=list(range(NCORES)), **kwargs)
    out = np.concatenate([res.results[i]["out"] for i in range(NCORES)], axis=0)
    return out.astype(np.float32), res


def kernel(x, w_qkv, b_qkv, w_proj, b_proj, num_t, num_s):
    assert int(num_t) == NT and int(num_s) == NS
    out, _ = _run(x, w_qkv, b_qkv, w_proj, b_proj)
    return out


# revision 22
# speedup vs baseline: 1.0826x; 1.0702x over previous
"""Sparse attention (template/search) Trainium2 Bass kernel.

Reference computation (B=64, N=320, C=768, H=12, D=64, num_t=64, num_s=256):
    qkv = x @ w_qkv.T + b_qkv           -> split to q, k, v per head
    template tokens 0:64   attend to tokens 0:64
    search   tokens 64:320 attend to all 320 tokens
    out = attn_out @ w_proj.T + b_proj

Data-parallel over batch across 8 NeuronCores (8 batches each). Host does all
layout transposes and dtype casts (bf16), plus two exact algebraic folds:
  - v-bias passes through softmax unchanged (rows sum to 1), so b_v is folded
    into an effective proj bias: b_proj_eff = b_proj + w_proj @ b_v.
  - b_proj_eff is pre-broadcast to [128, C] so the proj PSUM evacuation is a
    single tensor_tensor ADD (no rank-1 bias matmuls on the PE).
On-device dataflow per (batch, head):
  STk   = kT[d, kchunk].T @ qT[d, :]        (scores transposed, k on partitions,
                                             head pairs run row-group concurrent)
  PT    = exp(STk * 0.125)                  (ScalarE, PSUM -> SBUF, bf16)
  PV    = vaug[k, 65].T @ PT[k, q]          -> [65, 320]: rows 0:64 = attn outT,
                                               row 64 = colsums (ones column,
                                               written by a strided memset)
Softmax normalization is matmul-free: the 12 colsum rows are DMA-gathered into
a [128, 30] tile (one batched reciprocal at 8 cycles/element spread over 128
partitions), DMA-scattered back, then DMA-broadcast (free-dim stride-0 source)
into a [128, 1920] tile whose partition halves match the even/odd head packing
of the attention-out tiles; normalization is then 6 bf16 tensor_muls per batch.
Projection: out[t, co] = aT[c, t].T @ w_projT[c, co]; bias added during PSUM
evacuation via the pre-broadcast bias tile.
All matmul operands are bf16 (full PE rate at any moving size, FWL weight
loads); PSUM accumulation stays fp32. The v-projection's 64-token tail chunks
of the two batches in a pair are packed into one [128, 128] stationary (tails
DMA'd twice into a dedicated tile) so those matmuls use the full PE width.
The schedule is software-pipelined as in the baseline: pair p+1's dense qkv
matmuls are interleaved into pair p's attention phase to keep PE duty above
the HAM clock-gate threshold; weight DMAs are split (wqk in column halves) and
ordered so the first qkv matmul can start ~10us into the kernel.
"""

import sys

sys.path.insert(0, "/opt/trn_rl_repo")

import numpy as np
import ml_dtypes

B, N, C = 64, 320, 768
H, D = 12, 64
NT, NS = 64, 256
NCORES = 8
BC = B // NCORES  # batches per core
CCH = C // 128  # 6 contraction chunks
QK_TILES = (2 * C) // 128  # 12 co-tiles covering q and k sections
TCH = [(0, 128), (128, 128), (256, 64)]  # token chunks (t or k)
VW = H * 65  # 780: v width incl. ones columns
NPH = VW // 2  # 390: vnat free-dim half
PH = C // 2  # 384: proj free-dim half

_CACHE = {}


def _build():
    import concourse.bacc as bacc
    import concourse.mybir as mybir
    import concourse.tile as tile

    F32 = mybir.dt.float32
    BF16 = mybir.dt.bfloat16
    EXP = mybir.ActivationFunctionType.Exp

    nc = bacc.Bacc("TRN2")

    d_xt = nc.dram_tensor("xt", [BC, C, N], BF16, kind="ExternalInput")
    d_wqk = nc.dram_tensor("wqk", [C, 2 * C], BF16, kind="ExternalInput")
    d_wv = nc.dram_tensor("wv", [C, VW], BF16, kind="ExternalInput")
    d_wp = nc.dram_tensor("wp", [C, C], BF16, kind="ExternalInput")
    d_bqk = nc.dram_tensor("bqk", [128, QK_TILES], F32, kind="ExternalInput")
    d_bp = nc.dram_tensor("bp", [128, C], BF16, kind="ExternalInput")
    d_out = nc.dram_tensor("out", [BC, N, C], F32, kind="ExternalOutput")

    with tile.TileContext(nc) as tc:
        with (
            tc.tile_pool(name="const", bufs=1) as cp,
            tc.tile_pool(name="work", bufs=2) as wp,
            tc.tile_pool(name="psum", bufs=2, space="PSUM") as pp,
        ):
            # ---- resident weights; DMA order = first-use order ----
            bqk_sb = cp.tile([128, QK_TILES], F32, name="bqk", tag="bqk")
            nc.sync.dma_start(bqk_sb[:], d_bqk[:])

            def xt_dma(p):
                # one DMA per batch: [C, N] HBM -> [128, 6*N] SBUF (c-major
                # free dim), so the Sync engine issues 2 big DMAs instead of
                # 12 small ones (each dma_start costs ~650ns of issue time)
                xt_sb = {}
                bt = {}
                for b in (2 * p, 2 * p + 1):
                    t_xt = wp.tile(
                        [128, CCH * N], BF16, name=f"xt{b}", tag="xt", bufs=4
                    )
                    src = d_xt[b, :, :].rearrange("(c p) q -> p c q", p=128)
                    dst = t_xt[:, :].rearrange("p (c q) -> p c q", q=N)
                    nc.sync.dma_start(dst, src)
                    bt[b] = t_xt
                    for c in range(CCH):
                        xt_sb[(b, c)] = t_xt[:, c * N : (c + 1) * N]
                # tail tokens of both batches packed [b0 256:320 | b1 256:320]
                # per c-chunk, via 2 SBUF->SBUF DMAs
                t_tl = wp.tile([128, CCH * 128], BF16, name=f"xtl{p}", tag="xtl", bufs=2)
                for i, b in enumerate((2 * p, 2 * p + 1)):
                    src = bt[b][:, :].rearrange("p (c q) -> p c q", q=N)[:, :, 256:N]
                    dst = t_tl[:, :].rearrange("p (c i q) -> p c i q", i=2, q=64)[
                        :, :, i, :
                    ]
                    nc.sync.dma_start(dst, src)
                for c in range(CCH):
                    xt_sb[("tl", c)] = t_tl[:, c * 128 : (c + 1) * 128]
                return xt_sb

            pair_state = {0: {}}
            pair_state[0]["xt"] = xt_dma(0)

            wqk_sb = {}
            for hf in range(2):
                for c in range(CCH):
                    t_wqk = cp.tile(
                        [128, C], BF16, name=f"wqk{c}_{hf}", tag=f"wqk{c}_{hf}"
                    )
                    nc.sync.dma_start(
                        t_wqk[:], d_wqk[c * 128 : (c + 1) * 128, hf * C : (hf + 1) * C]
                    )
                    wqk_sb[(c, hf)] = t_wqk
            wv_sb = []
            wp_sb = []
            for c in range(CCH):
                t_wv = cp.tile([128, VW], BF16, name=f"wv{c}", tag=f"wv{c}")
                nc.sync.dma_start(t_wv[:], d_wv[c * 128 : (c + 1) * 128, :])
                wv_sb.append(t_wv)
            bp_sb = cp.tile([128, C], BF16, name="bp", tag="bp")
            nc.sync.dma_start(bp_sb[:], d_bp[:])
            for c in range(CCH):
                t_wp = cp.tile([128, C], BF16, name=f"wp{c}", tag=f"wp{c}")
                nc.sync.dma_start(t_wp[:], d_wp[c * 128 : (c + 1) * 128, :])
                wp_sb.append(t_wp)

            def attn_headpair(b, hp, qk_sb, vaug_sb, at_sb, sumsf):
                # head pair (2hp, 2hp+1): even head at partitions 0:64, odd
                # at 64:128 of the same qk tiles. The two score matmuls of a
                # chunk hit different PE row groups and run concurrently.
                qt = qk_sb[hp]
                kt = qk_sb[6 + hp]
                pt_sb = {0: [], 1: []}
                for ki, (k0, kl) in enumerate(TCH):
                    q0 = 0 if ki == 0 else 64
                    ps_pair = []
                    for par in range(2):
                        off = par * 64
                        ps = pp.tile(
                            [kl, N - q0],
                            F32,
                            name=f"pst{b}_{hp}_{par}_{ki}",
                            tag="pst",
                            bufs=3,
                        )
                        nc.tensor.matmul(
                            ps[:],
                            kt[off : off + 64, k0 : k0 + kl],
                            qt[off : off + 64, q0:N],
                            start=True,
                            stop=True,
                        )
                        ps_pair.append(ps)
                    for par in range(2):
                        t_pt = wp.tile(
                            [kl, N - q0],
                            BF16,
                            name=f"pt{b}_{hp}_{par}_{ki}",
                            tag="pt",
                            bufs=8,
                        )
                        nc.scalar.activation(t_pt[:], ps_pair[par][:], EXP, scale=0.125)
                        pt_sb[par].append(t_pt)
                for par in range(2):
                    h = 2 * hp + par
                    off = par * 64
                    pts = pt_sb[par]
                    # PV: rows 0:64 = attn outT (unnormalized), row 64 = colsums
                    po = pp.tile([65, N], F32, name=f"po{b}_{h}", tag="po", bufs=2)
                    hs = slice(h * 65, (h + 1) * 65)
                    nc.tensor.matmul(
                        po[:, 0:64],
                        vaug_sb[0][0:64, hs],
                        pts[0][0:64, 0:64],
                        start=True,
                        stop=False,
                    )
                    nc.tensor.matmul(
                        po[:, 64:N],
                        vaug_sb[0][:, hs],
                        pts[0][:, 64:N],
                        start=False,
                        stop=False,
                    )
                    nc.tensor.matmul(
                        po[:, 64:N], vaug_sb[1][:, hs], pts[1][:], start=False, stop=False
                    )
                    nc.tensor.matmul(
                        po[:, 64:N], vaug_sb[2][:, hs], pts[2][:], start=False, stop=True
                    )
                    # evacuate unnormalized rows + colsum row; frees the bank.
                    # sums go to the half-batch tile (head pairs 0:3 / 3:6),
                    # par-major [par*960 + (hp%3)*320 + q] so the broadcast
                    # sources are contiguous per parity.
                    nc.any.tensor_copy(at_sb[hp][off : off + 64, :], po[0:64, :])
                    so = par * (3 * N) + (hp % 3) * N
                    sf = sumsf[0] if hp < 3 else sumsf[1]
                    nc.any.tensor_copy(sf[0:1, so : so + N], po[64:65, :])

            def attn_chain(b, sf, half):
                # batched softmax reciprocals for one half-batch (3 head
                # pairs): gather the 6*N sums across 64 partitions (DVE
                # reciprocal costs 8 cycles/elem serially per partition),
                # scatter back flat, then DMA-broadcast (free-dim stride-0
                # source) to a [128, 3*N] tile whose partition halves match
                # the even/odd head packing of the attention-out tiles.
                # Split in halves so normalization can start 3 head pairs
                # earlier and the chain latency hides behind attention.
                HN = 3 * N
                s64 = wp.tile([64, 30], BF16, name=f"s64_{b}_{half}", tag="s64", bufs=2)
                nc.sync.dma_start(
                    s64[:, :], sf[0:1, :].rearrange("o (p q) -> o p q", p=64)
                )
                rr = wp.tile([64, 30], BF16, name=f"rr{b}_{half}", tag="rr", bufs=2)
                with nc.allow_low_precision(reason="bf16 softmax reciprocal"):
                    nc.vector.reciprocal(rr[:], s64[:])
                rcpf = wp.tile([1, 2 * HN], BF16, name=f"rcpf{b}_{half}", tag="rcpf", bufs=2)
                nc.sync.dma_start(
                    rcpf[0:1, :].rearrange("o (p q) -> o p q", p=64), rr[:, :]
                )
                bc = wp.tile([128, HN], BF16, name=f"bc{b}_{half}", tag="bc", bufs=4)
                for par in range(2):
                    src = (
                        rcpf[0:1, par * HN : (par + 1) * HN]
                        .rearrange("o (b q) -> o b q", b=1)
                        .broadcast_to([1, 64, HN])
                    )
                    nc.sync.dma_start(bc[par * 64 : (par + 1) * 64, :], src)
                return bc

            def norm_half(b, at_sb, bc, half):
                for hp in range(3 * half, 3 * half + 3):
                    with nc.allow_low_precision(reason="bf16 attn normalize"):
                        nc.vector.tensor_mul(
                            at_sb[hp][:, :],
                            at_sb[hp][:, :],
                            bc[:, (hp % 3) * N : (hp % 3 + 1) * N],
                        )

            def proj_unit(b, ti, at_sb):
                # nh-outer so each accumulation chain holds a single PSUM
                # slot: a late evacuation then stalls only one chain, not the
                # next unit's matmuls
                t0, tl = TCH[ti]
                t_o = wp.tile([tl, C], BF16, name=f"outp{b}_{ti}", tag="outp", bufs=3)
                for nh in range(2):
                    ps = pp.tile(
                        [tl, PH], F32, name=f"psp{b}_{ti}_{nh}", tag="pmm", bufs=3
                    )
                    for c in range(CCH):
                        nc.tensor.matmul(
                            ps[:],
                            at_sb[c][:, t0 : t0 + tl],
                            wp_sb[c][:, nh * PH : (nh + 1) * PH],
                            start=(c == 0),
                            stop=(c == CCH - 1),
                        )
                    with nc.allow_low_precision(reason="bf16 out staging"):
                        nc.vector.tensor_add(
                            t_o[:, nh * PH : (nh + 1) * PH],
                            ps[:],
                            bp_sb[0:tl, nh * PH : (nh + 1) * PH],
                        )
                # gpsimd-initiated DMA widens bf16 -> fp32 on the way out
                nc.gpsimd.dma_start(d_out[b, t0 : t0 + tl, :], t_o[:])

            def _vnat_mms(ps, stat_of_c, nh):
                for c in range(CCH):
                    nc.tensor.matmul(
                        ps[:],
                        stat_of_c(c),
                        wv_sb[c][:, nh * NPH : (nh + 1) * NPH],
                        start=(c == 0),
                        stop=(c == CCH - 1),
                    )

            def _vnat_ones(t_v):
                ones_ap = t_v[:, :].rearrange("p (h c) -> p h c", c=65)[:, :, 64:65]
                nc.gpsimd.memset(ones_ap, 1.0)

            def vnat_unit(b, ti, xt_sb):
                # head chunks 0/1 of one batch: [128, VW] stationary
                t0, tl = TCH[ti]
                t_v = wp.tile([tl, VW], BF16, name=f"vaug{b}_{ti}", tag="vaug", bufs=8)
                for nh in range(2):
                    ps = pp.tile(
                        [tl, NPH], F32, name=f"psv{b}_{ti}_{nh}", tag="pmm", bufs=3
                    )
                    _vnat_mms(ps, lambda c: xt_sb[(b, c)][:, t0 : t0 + tl], nh)
                    nc.any.tensor_copy(t_v[:, nh * NPH : (nh + 1) * NPH], ps[:])
                _vnat_ones(t_v)
                return t_v

            def vnat_tail(p, xt_sb):
                # both batches' 64-token tails in one [128, 128] stationary
                t_v0 = wp.tile([64, VW], BF16, name=f"vaugt{2*p}", tag="vaug", bufs=8)
                t_v1 = wp.tile([64, VW], BF16, name=f"vaugt{2*p+1}", tag="vaug", bufs=8)
                for nh in range(2):
                    ps = pp.tile(
                        [128, NPH], F32, name=f"psvt{p}_{nh}", tag="pmm", bufs=3
                    )
                    _vnat_mms(ps, lambda c: xt_sb[("tl", c)], nh)
                    nc.any.tensor_copy(t_v0[:, nh * NPH : (nh + 1) * NPH], ps[0:64, :])
                    nc.any.tensor_copy(t_v1[:, nh * NPH : (nh + 1) * NPH], ps[64:128, :])
                _vnat_ones(t_v0)
                _vnat_ones(t_v1)
                return t_v0, t_v1

            def qkv_unit(p, j, b, xt_sb, qk_sb):
                # qkT projection for one (co-tile, batch): a single PSUM slot
                # per accumulation chain
                hf, jc = j // 6, j % 6
                ps = pp.tile([128, N], F32, name=f"psqk{b}_{j}", tag="pmm", bufs=3)
                for c in range(CCH):
                    nc.tensor.matmul(
                        ps[:],
                        wqk_sb[(c, hf)][:, jc * 128 : (jc + 1) * 128],
                        xt_sb[(b, c)],
                        start=(c == 0),
                        stop=(c == CCH - 1),
                    )
                t_qk = wp.tile([128, N], BF16, name=f"qk{b}_{j}", tag="qkt", bufs=50)
                with nc.allow_low_precision(reason="bf16 q/k for scores"):
                    nc.vector.tensor_scalar_add(t_qk[:], ps[:], bqk_sb[:, j : j + 1])
                qk_sb[b].append(t_qk)

            def emit_pair(p, qk_sb, filler, vnat_hook):
                """attention + normalize + projection for pair p. filler()
                injects one of the next pair's dense qkv units; vnat_hook()
                injects the next pair's v-projection units into the shadow of
                b1's reciprocal DMA chain. b0's normalize+projection folds
                into b1's attention so the PE never head-of-line blocks on
                b0's chain. All to keep the PE duty above the HAM clock-gate
                threshold (the PE runs at 1.2 GHz instead of 2.4 when its
                duty drops for a ~3.4us window)."""
                b0, b1 = 2 * p, 2 * p + 1
                xt_sb = pair_state[p]["xt"]
                V = pair_state[p]["vaug"]
                vaug0 = [V[0], V[1], V["t0"]]
                at0 = [
                    wp.tile([128, N], BF16, name=f"at{b0}_{j}", tag="at", bufs=18)
                    for j in range(CCH)
                ]
                sumsf0 = [
                    wp.tile([1, H * N // 2], BF16, name=f"sumsf{b0}_{h}", tag="sumsf", bufs=4)
                    for h in range(2)
                ]
                vaug1 = [None, None, V["t1"]]
                bc0 = [None, None]
                for hp in range(H // 2):
                    attn_headpair(b0, hp, qk_sb[b0], vaug0, at0, sumsf0)
                    if hp == 0:
                        vaug1[0] = vnat_unit(b1, 0, xt_sb)
                    elif hp == 2:
                        bc0[0] = attn_chain(b0, sumsf0[0], 0)
                        vaug1[1] = vnat_unit(b1, 1, xt_sb)
                    else:
                        filler()
                bc0[1] = attn_chain(b0, sumsf0[1], 1)
                at1 = [
                    wp.tile([128, N], BF16, name=f"at{b1}_{j}", tag="at", bufs=18)
                    for j in range(CCH)
                ]
                sumsf1 = [
                    wp.tile([1, H * N // 2], BF16, name=f"sumsf{b1}_{h}", tag="sumsf", bufs=4)
                    for h in range(2)
                ]
                bc1 = [None, None]
                for hp in range(H // 2):
                    attn_headpair(b1, hp, qk_sb[b1], vaug1, at1, sumsf1)
                    if hp == 0:
                        filler()
                    elif hp == 1:
                        norm_half(b0, at0, bc0[0], 0)
                    elif hp == 2:
                        bc1[0] = attn_chain(b1, sumsf1[0], 0)
                        filler()
                    elif hp == 3:
                        norm_half(b0, at0, bc0[1], 1)
                    elif hp == 4:
                        proj_unit(b0, 0, at0)
                    elif hp == 5:
                        proj_unit(b0, 1, at0)
                filler()
                proj_unit(b0, 2, at0)
                bc1[1] = attn_chain(b1, sumsf1[1], 1)
                vnat_hook()
                norm_half(b1, at1, bc1[0], 0)
                filler()
                norm_half(b1, at1, bc1[1], 1)
                filler()
                proj_unit(b1, 0, at1)
                filler()
                proj_unit(b1, 1, at1)
                filler()
                proj_unit(b1, 2, at1)
                filler()

            def emit_vnat_pair(p):
                xt_sb = pair_state[p]["xt"]
                vt0, vt1 = vnat_tail(p, xt_sb)
                V = {"t0": vt0, "t1": vt1}
                V[0] = vnat_unit(2 * p, 0, xt_sb)
                V[1] = vnat_unit(2 * p, 1, xt_sb)
                pair_state[p]["vaug"] = V

            # ---- software-pipelined driver: pair p+1's dense qkv and
            # v-projection matmuls are emitted interleaved into pair p's
            # attention/projection phase, paced evenly over the filler sites
            # so the PE never sees a long matmul-free window ----
            NP = BC // 2
            for p in range(NP):
                pair_state.setdefault(p, {})
                pair_state[p]["qk"] = {2 * p: [], 2 * p + 1: []}
            for j in range(QK_TILES):
                for b in (0, 1):
                    qkv_unit(0, j, b, pair_state[0]["xt"], pair_state[0]["qk"])
            emit_vnat_pair(0)
            N_SITES = 12  # filler() call sites per emit_pair
            for p in range(NP):
                if p + 1 < NP:
                    pair_state[p + 1]["xt"] = xt_dma(p + 1)
                    nxt = pair_state[p + 1]
                    units = [
                        (lambda j=j, b=b, pn=p + 1, nxt=nxt: qkv_unit(
                            pn, j, 2 * pn + b, nxt["xt"], nxt["qk"]
                        ))
                        for j in range(QK_TILES)
                        for b in (0, 1)
                    ]
                    st = {"site": 0, "done": 0}

                    def filler(units=units, st=st):
                        st["site"] += 1
                        tgt = st["site"] * len(units) // N_SITES
                        while st["done"] < min(tgt, len(units)):
                            units[st["done"]]()
                            st["done"] += 1

                    def vnat_hook(pn=p + 1):
                        emit_vnat_pair(pn)
                else:

                    def filler():
                        pass

                    def vnat_hook():
                        pass
                emit_pair(p, pair_state[p]["qk"], filler, vnat_hook)

    nc.compile()
    return nc


def _get_nc():
    if "nc" not in _CACHE:
        _CACHE["nc"] = _build()
    return _CACHE["nc"]


def _host_prep(x, w_qkv, b_qkv, w_proj, b_proj):
    x = np.asarray(x, dtype=np.float32)
    w_qkv = np.asarray(w_qkv, dtype=np.float32)
    b_qkv = np.asarray(b_qkv, dtype=np.float32)
    w_proj = np.asarray(w_proj, dtype=np.float32)
    b_proj = np.asarray(b_proj, dtype=np.float32)
    bf16 = ml_dtypes.bfloat16

    xt = np.ascontiguousarray(x.transpose(0, 2, 1)).astype(bf16)  # [B, C, N]
    wqk = np.ascontiguousarray(w_qkv[: 2 * C].T).astype(bf16)  # [C, 2C]
    wv_nat = w_qkv[2 * C :]  # [C(hd), C(c)]
    wv = np.zeros((C, VW), dtype=np.float32)
    for h in range(H):
        wv[:, h * 65 : h * 65 + 64] = wv_nat[h * 64 : (h + 1) * 64].T
    wv = wv.astype(bf16)
    bqk = np.ascontiguousarray(b_qkv[: 2 * C].reshape(QK_TILES, 128).T)  # [128, 12]
    wpr = np.ascontiguousarray(w_proj.T).astype(bf16)  # [C, C]
    # v-bias passes through softmax (rows sum to 1): fold into proj bias,
    # then pre-broadcast to [128, C] for the tensor_tensor bias add.
    bp_eff = b_proj + w_proj @ b_qkv[2 * C :]
    bp = np.broadcast_to(bp_eff.reshape(1, C), (128, C)).astype(bf16)
    bp = np.ascontiguousarray(bp)
    return xt, wqk, wv, wpr, bqk, bp


def _run(x, w_qkv, b_qkv, w_proj, b_proj, trace=False, trace_cores=None):
    from concourse.bass_utils import run_bass_kernel_spmd

    xt, wqk, wv, wpr, bqk, bp = _host_prep(x, w_qkv, b_qkv, w_proj, b_proj)
    nc = _get_nc()
    in_maps = []
    for i in range(NCORES):
        in_maps.append(
            {
                "xt": xt[i * BC : (i + 1) * BC],
                "wqk": wqk,
                "wv": wv,
                "wp": wpr,
                "bqk": bqk,
                "bp": bp,
            }
        )
    kwargs = {}
    if trace:
        kwargs = {"trace": True, "trace_cores": trace_cores or [0]}
    res = run_bass_kernel_spmd(nc, in_maps, core_ids=list(range(NCORES)), **kwargs)
    out = np.concatenate([res.results[i]["out"] for i in range(NCORES)], axis=0)
    return out.astype(np.float32), res


def kernel(x, w_qkv, b_qkv, w_proj, b_proj, num_t, num_s):
    assert int(num_t) == NT and int(num_s) == NS
    out, _ = _run(x, w_qkv, b_qkv, w_proj, b_proj)
    return out


# revision 25
# speedup vs baseline: 1.1726x; 1.0831x over previous
"""Sparse attention (template/search) Trainium2 Bass kernel.

Reference computation (B=64, N=320, C=768, H=12, D=64, num_t=64, num_s=256):
    qkv = x @ w_qkv.T + b_qkv           -> split to q, k, v per head
    template tokens 0:64   attend to tokens 0:64
    search   tokens 64:320 attend to all 320 tokens
    out = attn_out @ w_proj.T + b_proj

Data-parallel over batch across 8 NeuronCores (8 batches each). Host does all
layout transposes and dtype casts (bf16), plus two exact algebraic folds:
  - v-bias passes through softmax unchanged (rows sum to 1), so b_v is folded
    into an effective proj bias: b_proj_eff = b_proj + w_proj @ b_v.
  - b_proj_eff is pre-broadcast to [128, C] so the proj PSUM evacuation is a
    single tensor_tensor ADD (no rank-1 bias matmuls on the PE).
On-device dataflow per (batch, head):
  STk   = kT[d, kchunk].T @ qT[d, :]        (scores transposed, k on partitions,
                                             head pairs run row-group concurrent)
  PT    = exp(STk * 0.125)                  (ScalarE, PSUM -> SBUF, bf16)
  PV    = vaug[k, 65].T @ PT[k, q]          -> [65, 320]: rows 0:64 = attn outT,
                                               row 64 = colsums (ones column,
                                               written by a strided memset)
Softmax normalization is matmul-free: the 12 colsum rows are DMA-gathered into
a [128, 30] tile (one batched reciprocal at 8 cycles/element spread over 128
partitions), DMA-scattered back, then DMA-broadcast (free-dim stride-0 source)
into a [128, 1920] tile whose partition halves match the even/odd head packing
of the attention-out tiles; normalization is then 6 bf16 tensor_muls per batch.
Projection: out[t, co] = aT[c, t].T @ w_projT[c, co]; bias added during PSUM
evacuation via the pre-broadcast bias tile.
All matmul operands are bf16 (full PE rate at any moving size, FWL weight
loads); PSUM accumulation stays fp32. The v-projection's 64-token tail chunks
of the two batches in a pair are packed into one [128, 128] stationary (tails
DMA'd twice into a dedicated tile) so those matmuls use the full PE width.
The schedule is software-pipelined as in the baseline: pair p+1's dense qkv
matmuls are interleaved into pair p's attention phase to keep PE duty above
the HAM clock-gate threshold; weight DMAs are split (wqk in column halves) and
ordered so the first qkv matmul can start ~10us into the kernel.
"""

import sys

sys.path.insert(0, "/opt/trn_rl_repo")

import numpy as np
import ml_dtypes

B, N, C = 64, 320, 768
H, D = 12, 64
NT, NS = 64, 256
NCORES = 8
BC = B // NCORES  # batches per core
CCH = C // 128  # 6 contraction chunks
QK_TILES = (2 * C) // 128  # 12 co-tiles covering q and k sections
TCH = [(0, 128), (128, 128), (256, 64)]  # token chunks (t or k)
VW = H * 65  # 780: v width incl. ones columns
NPH = VW // 2  # 390: vnat free-dim half
PH = C // 2  # 384: proj free-dim half

_CACHE = {}


def _build():
    import concourse.bacc as bacc
    import concourse.mybir as mybir
    import concourse.tile as tile

    F32 = mybir.dt.float32
    BF16 = mybir.dt.bfloat16
    EXP = mybir.ActivationFunctionType.Exp

    nc = bacc.Bacc("TRN2")

    d_xt = nc.dram_tensor("xt", [BC, C, N], BF16, kind="ExternalInput")
    d_wqk = nc.dram_tensor("wqk", [C, 2 * C], BF16, kind="ExternalInput")
    d_wv = nc.dram_tensor("wv", [C, VW], BF16, kind="ExternalInput")
    d_wp = nc.dram_tensor("wp", [C, C], BF16, kind="ExternalInput")
    d_bqk = nc.dram_tensor("bqk", [128, QK_TILES], F32, kind="ExternalInput")
    d_bp = nc.dram_tensor("bp", [128, C], BF16, kind="ExternalInput")
    d_out = nc.dram_tensor("out", [BC, N, C], F32, kind="ExternalOutput")

    with tile.TileContext(nc) as tc:
        with (
            tc.tile_pool(name="const", bufs=1) as cp,
            tc.tile_pool(name="work", bufs=2) as wp,
            tc.tile_pool(name="psum", bufs=2, space="PSUM") as pp,
        ):
            # ---- resident weights; DMA order = first-use order ----
            bqk_sb = cp.tile([128, QK_TILES], F32, name="bqk", tag="bqk")
            nc.sync.dma_start(bqk_sb[:], d_bqk[:])

            def xt_dma(p):
                # one DMA per batch: [C, N] HBM -> [128, 6*N] SBUF (c-major
                # free dim), so the Sync engine issues 2 big DMAs instead of
                # 12 small ones (each dma_start costs ~650ns of issue time)
                xt_sb = {}
                bt = {}
                for b in (2 * p, 2 * p + 1):
                    t_xt = wp.tile(
                        [128, CCH * N], BF16, name=f"xt{b}", tag="xt", bufs=4
                    )
                    src = d_xt[b, :, :].rearrange("(c p) q -> p c q", p=128)
                    dst = t_xt[:, :].rearrange("p (c q) -> p c q", q=N)
                    nc.sync.dma_start(dst, src)
                    bt[b] = t_xt
                    for c in range(CCH):
                        xt_sb[(b, c)] = t_xt[:, c * N : (c + 1) * N]
                # tail tokens of both batches packed [b0 256:320 | b1 256:320]
                # per c-chunk, via 2 SBUF->SBUF DMAs
                t_tl = wp.tile([128, CCH * 128], BF16, name=f"xtl{p}", tag="xtl", bufs=2)
                for i, b in enumerate((2 * p, 2 * p + 1)):
                    src = bt[b][:, :].rearrange("p (c q) -> p c q", q=N)[:, :, 256:N]
                    dst = t_tl[:, :].rearrange("p (c i q) -> p c i q", i=2, q=64)[
                        :, :, i, :
                    ]
                    nc.sync.dma_start(dst, src)
                for c in range(CCH):
                    xt_sb[("tl", c)] = t_tl[:, c * 128 : (c + 1) * 128]
                return xt_sb

            pair_state = {0: {}}
            pair_state[0]["xt"] = xt_dma(0)

            wqk_sb = {}
            for hf in range(2):
                for c in range(CCH):
                    t_wqk = cp.tile(
                        [128, C], BF16, name=f"wqk{c}_{hf}", tag=f"wqk{c}_{hf}"
                    )
                    nc.sync.dma_start(
                        t_wqk[:], d_wqk[c * 128 : (c + 1) * 128, hf * C : (hf + 1) * C]
                    )
                    wqk_sb[(c, hf)] = t_wqk
            wv_sb = []
            wp_sb = []
            for c in range(CCH):
                t_wv = cp.tile([128, VW], BF16, name=f"wv{c}", tag=f"wv{c}")
                nc.sync.dma_start(t_wv[:], d_wv[c * 128 : (c + 1) * 128, :])
                wv_sb.append(t_wv)
            bp_sb = cp.tile([128, C], BF16, name="bp", tag="bp")
            nc.sync.dma_start(bp_sb[:], d_bp[:])
            for c in range(CCH):
                t_wp = cp.tile([128, C], BF16, name=f"wp{c}", tag=f"wp{c}")
                nc.sync.dma_start(t_wp[:], d_wp[c * 128 : (c + 1) * 128, :])
                wp_sb.append(t_wp)

            def attn_headpair(b, hp, qk_sb, vaug_sb, at_sb, sumsf):
                # head pair (2hp, 2hp+1): even head at partitions 0:64, odd
                # at 64:128 of the same qk tiles. The two score matmuls of a
                # chunk hit different PE row groups and run concurrently.
                qt = qk_sb[hp]
                kt = qk_sb[6 + hp]
                pt_sb = {0: [], 1: []}
                for ki, (k0, kl) in enumerate(TCH):
                    q0 = 0 if ki == 0 else 64
                    ps_pair = []
                    for par in range(2):
                        off = par * 64
                        ps = pp.tile(
                            [kl, N - q0],
                            F32,
                            name=f"pst{b}_{hp}_{par}_{ki}",
                            tag="pst",
                            bufs=3,
                        )
                        nc.tensor.matmul(
                            ps[:],
                            kt[off : off + 64, k0 : k0 + kl],
                            qt[off : off + 64, q0:N],
                            start=True,
                            stop=True,
                        )
                        ps_pair.append(ps)
                    for par in range(2):
                        t_pt = wp.tile(
                            [kl, N - q0],
                            BF16,
                            name=f"pt{b}_{hp}_{par}_{ki}",
                            tag="pt",
                            bufs=8,
                        )
                        nc.scalar.activation(t_pt[:], ps_pair[par][:], EXP, scale=0.125)
                        pt_sb[par].append(t_pt)
                for par in range(2):
                    h = 2 * hp + par
                    off = par * 64
                    pts = pt_sb[par]
                    # PV: rows 0:64 = attn outT (unnormalized), row 64 = colsums
                    po = pp.tile([65, N], F32, name=f"po{b}_{h}", tag="po", bufs=2)
                    hs = slice(h * 65, (h + 1) * 65)
                    nc.tensor.matmul(
                        po[:, 0:64],
                        vaug_sb[0][0:64, hs],
                        pts[0][0:64, 0:64],
                        start=True,
                        stop=False,
                    )
                    nc.tensor.matmul(
                        po[:, 64:N],
                        vaug_sb[0][:, hs],
                        pts[0][:, 64:N],
                        start=False,
                        stop=False,
                    )
                    nc.tensor.matmul(
                        po[:, 64:N], vaug_sb[1][:, hs], pts[1][:], start=False, stop=False
                    )
                    nc.tensor.matmul(
                        po[:, 64:N], vaug_sb[2][:, hs], pts[2][:], start=False, stop=True
                    )
                    # evacuate unnormalized rows + colsum row; frees the bank.
                    # sums go to the half-batch tile (head pairs 0:3 / 3:6),
                    # par-major [par*960 + (hp%3)*320 + q] so the broadcast
                    # sources are contiguous per parity.
                    # sums first: the reciprocal DMA chain hangs off them, so
                    # they must not queue behind the fat at-row evacuations
                    so = par * (3 * N) + (hp % 3) * N
                    sf = sumsf[0] if hp < 3 else sumsf[1]
                    nc.any.tensor_copy(sf[0:1, so : so + N], po[64:65, :])
                    nc.any.tensor_copy(at_sb[hp][off : off + 64, :], po[0:64, :])

            def attn_chain(b, sf, half):
                # batched softmax reciprocals for one half-batch (3 head
                # pairs): gather the 6*N sums across 64 partitions (DVE
                # reciprocal costs 8 cycles/elem serially per partition),
                # scatter back flat, then DMA-broadcast (free-dim stride-0
                # source) to a [128, 3*N] tile whose partition halves match
                # the even/odd head packing of the attention-out tiles.
                # Split in halves so normalization can start 3 head pairs
                # earlier and the chain latency hides behind attention.
                HN = 3 * N
                s64 = wp.tile([64, 30], BF16, name=f"s64_{b}_{half}", tag="s64", bufs=2)
                nc.sync.dma_start(
                    s64[:, :], sf[0:1, :].rearrange("o (p q) -> o p q", p=64)
                )
                rr = wp.tile([64, 30], BF16, name=f"rr{b}_{half}", tag="rr", bufs=2)
                with nc.allow_low_precision(reason="bf16 softmax reciprocal"):
                    nc.vector.reciprocal(rr[:], s64[:])
                rcpf = wp.tile([1, 2 * HN], BF16, name=f"rcpf{b}_{half}", tag="rcpf", bufs=2)
                nc.sync.dma_start(
                    rcpf[0:1, :].rearrange("o (p q) -> o p q", p=64), rr[:, :]
                )
                bc = wp.tile([128, HN], BF16, name=f"bc{b}_{half}", tag="bc", bufs=6)
                for par in range(2):
                    src = (
                        rcpf[0:1, par * HN : (par + 1) * HN]
                        .rearrange("o (b q) -> o b q", b=1)
                        .broadcast_to([1, 64, HN])
                    )
                    nc.sync.dma_start(bc[par * 64 : (par + 1) * 64, :], src)
                return bc

            def norm_half(b, at_sb, bc, half):
                for hp in range(3 * half, 3 * half + 3):
                    with nc.allow_low_precision(reason="bf16 attn normalize"):
                        nc.vector.tensor_mul(
                            at_sb[hp][:, :],
                            at_sb[hp][:, :],
                            bc[:, (hp % 3) * N : (hp % 3 + 1) * N],
                        )

            def proj_unit(b, ti, at_sb):
                # nh-outer so each accumulation chain holds a single PSUM
                # slot: a late evacuation then stalls only one chain, not the
                # next unit's matmuls
                t0, tl = TCH[ti]
                t_o = wp.tile([tl, C], BF16, name=f"outp{b}_{ti}", tag="outp", bufs=3)
                for nh in range(2):
                    ps = pp.tile(
                        [tl, PH], F32, name=f"psp{b}_{ti}_{nh}", tag="pmm", bufs=3
                    )
                    for c in range(CCH):
                        nc.tensor.matmul(
                            ps[:],
                            at_sb[c][:, t0 : t0 + tl],
                            wp_sb[c][:, nh * PH : (nh + 1) * PH],
                            start=(c == 0),
                            stop=(c == CCH - 1),
                        )
                    with nc.allow_low_precision(reason="bf16 out staging"):
                        nc.vector.tensor_add(
                            t_o[:, nh * PH : (nh + 1) * PH],
                            ps[:],
                            bp_sb[0:tl, nh * PH : (nh + 1) * PH],
                        )
                # gpsimd-initiated DMA widens bf16 -> fp32 on the way out
                nc.gpsimd.dma_start(d_out[b, t0 : t0 + tl, :], t_o[:])

            def _vnat_mms(ps, stat_of_c, nh):
                for c in range(CCH):
                    nc.tensor.matmul(
                        ps[:],
                        stat_of_c(c),
                        wv_sb[c][:, nh * NPH : (nh + 1) * NPH],
                        start=(c == 0),
                        stop=(c == CCH - 1),
                    )

            def _vnat_ones(t_v):
                ones_ap = t_v[:, :].rearrange("p (h c) -> p h c", c=65)[:, :, 64:65]
                nc.gpsimd.memset(ones_ap, 1.0)

            def vnat_unit(b, ti, xt_sb):
                # head chunks 0/1 of one batch: [128, VW] stationary
                t0, tl = TCH[ti]
                t_v = wp.tile([tl, VW], BF16, name=f"vaug{b}_{ti}", tag="vaug", bufs=8)
                for nh in range(2):
                    ps = pp.tile(
                        [tl, NPH], F32, name=f"psv{b}_{ti}_{nh}", tag="pmm", bufs=3
                    )
                    _vnat_mms(ps, lambda c: xt_sb[(b, c)][:, t0 : t0 + tl], nh)
                    nc.any.tensor_copy(t_v[:, nh * NPH : (nh + 1) * NPH], ps[:])
                _vnat_ones(t_v)
                return t_v

            def vnat_tail(p, xt_sb):
                # both batches' 64-token tails in one [128, 128] stationary
                t_v0 = wp.tile([64, VW], BF16, name=f"vaugt{2*p}", tag="vaug", bufs=8)
                t_v1 = wp.tile([64, VW], BF16, name=f"vaugt{2*p+1}", tag="vaug", bufs=8)
                for nh in range(2):
                    ps = pp.tile(
                        [128, NPH], F32, name=f"psvt{p}_{nh}", tag="pmm", bufs=3
                    )
                    _vnat_mms(ps, lambda c: xt_sb[("tl", c)], nh)
                    nc.any.tensor_copy(t_v0[:, nh * NPH : (nh + 1) * NPH], ps[0:64, :])
                    nc.any.tensor_copy(t_v1[:, nh * NPH : (nh + 1) * NPH], ps[64:128, :])
                _vnat_ones(t_v0)
                _vnat_ones(t_v1)
                return t_v0, t_v1

            def qkv_unit(p, j, b, xt_sb, qk_sb):
                # qkT projection for one (co-tile, batch): a single PSUM slot
                # per accumulation chain
                hf, jc = j // 6, j % 6
                ps = pp.tile([128, N], F32, name=f"psqk{b}_{j}", tag="pmm", bufs=3)
                for c in range(CCH):
                    nc.tensor.matmul(
                        ps[:],
                        wqk_sb[(c, hf)][:, jc * 128 : (jc + 1) * 128],
                        xt_sb[(b, c)],
                        start=(c == 0),
                        stop=(c == CCH - 1),
                    )
                t_qk = wp.tile([128, N], BF16, name=f"qk{b}_{j}", tag="qkt", bufs=50)
                with nc.allow_low_precision(reason="bf16 q/k for scores"):
                    nc.vector.tensor_scalar_add(t_qk[:], ps[:], bqk_sb[:, j : j + 1])
                qk_sb[b].append(t_qk)

            def norm_proj_prev(prev, slot):
                # previous batch's normalize+projection, folded into the
                # current batch's attention phase: by the time these reach
                # the PE FIFO, the previous batch's reciprocal DMA chain has
                # had a full attention phase to complete, and the PE always
                # has attention matmuls queued ahead of them
                if prev is None:
                    return
                pb, pat, pbc = prev
                if slot == 0:
                    norm_half(pb, pat, pbc[0], 0)
                elif slot == 1:
                    norm_half(pb, pat, pbc[1], 1)
                else:
                    proj_unit(pb, slot - 2, pat)

            def emit_batch(b, qk_list, vaug, prev, hooks):
                """one batch's attention phase, with the previous batch's
                normalize+projection folded in and hooks() filling the PE
                with the next pair's dense qkv/v-projection matmuls to keep
                PE duty above the HAM clock-gate threshold (the PE runs at
                1.2 GHz instead of 2.4 when its duty drops for ~3.4us)."""
                at = [
                    wp.tile([128, N], BF16, name=f"at{b}_{j}", tag="at", bufs=18)
                    for j in range(CCH)
                ]
                sumsf = [
                    wp.tile([1, H * N // 2], BF16, name=f"sumsf{b}_{h}", tag="sumsf", bufs=6)
                    for h in range(2)
                ]
                bc = [None, None]
                for hp in range(H // 2):
                    attn_headpair(b, hp, qk_list, vaug, at, sumsf)
                    if hp == 1:
                        norm_proj_prev(prev, 0)
                    elif hp == 2:
                        bc[0] = attn_chain(b, sumsf[0], 0)
                    elif hp == 3:
                        norm_proj_prev(prev, 1)
                    elif hp == 4:
                        norm_proj_prev(prev, 2)
                    elif hp == 5:
                        norm_proj_prev(prev, 3)
                    hooks(hp)
                bc[1] = attn_chain(b, sumsf[1], 1)
                norm_proj_prev(prev, 4)
                hooks(6)
                return (b, at, bc)

            def emit_vnat_pair(p):
                xt_sb = pair_state[p]["xt"]
                vt0, vt1 = vnat_tail(p, xt_sb)
                V = {"t0": vt0, "t1": vt1}
                V[0] = vnat_unit(2 * p, 0, xt_sb)
                V[1] = vnat_unit(2 * p, 1, xt_sb)
                pair_state[p]["vaug"] = V

            # ---- software-pipelined driver, batch-granular: batch k's
            # normalize+projection folds into batch k+1's attention phase;
            # pair p+1's qkv/v-projection units are paced evenly over pair
            # p's filler sites so the PE never sees a long matmul-free
            # window ----
            NP = BC // 2
            for p in range(NP):
                pair_state.setdefault(p, {})
                pair_state[p]["qk"] = {2 * p: [], 2 * p + 1: []}
            for j in range(QK_TILES):
                for b in (0, 1):
                    qkv_unit(0, j, b, pair_state[0]["xt"], pair_state[0]["qk"])
            emit_vnat_pair(0)
            N_SITES = 8  # filler() call sites per pair
            prev = None
            for p in range(NP):
                if p + 1 < NP:
                    pair_state[p + 1]["xt"] = xt_dma(p + 1)
                    nxt = pair_state[p + 1]
                    units = [
                        (lambda j=j, b=b, pn=p + 1, nxt=nxt: qkv_unit(
                            pn, j, 2 * pn + b, nxt["xt"], nxt["qk"]
                        ))
                        for j in range(QK_TILES)
                        for b in (0, 1)
                    ]
                    units.append(lambda pn=p + 1: emit_vnat_pair(pn))
                    st = {"site": 0, "done": 0}

                    def filler(units=units, st=st):
                        st["site"] += 1
                        tgt = st["site"] * len(units) // N_SITES
                        while st["done"] < min(tgt, len(units)):
                            units[st["done"]]()
                            st["done"] += 1
                else:

                    def filler():
                        pass
                ba, bb = 2 * p, 2 * p + 1
                xt_sb = pair_state[p]["xt"]
                V = pair_state[p]["vaug"]
                vaug_a = [V[0], V[1], V["t0"]]
                vaug_b = [None, None, V["t1"]]

                def hooks_a(site, vaug_b=vaug_b, bb=bb, xt_sb=xt_sb, filler=filler):
                    if site == 0:
                        vaug_b[0] = vnat_unit(bb, 0, xt_sb)
                    elif site == 2:
                        vaug_b[1] = vnat_unit(bb, 1, xt_sb)
                    elif site in (1, 3, 6):
                        filler()

                def hooks_b(site, filler=filler):
                    if site in (0, 1, 2, 3, 6):
                        filler()

                prev = emit_batch(ba, pair_state[p]["qk"][ba], vaug_a, prev, hooks_a)
                prev = emit_batch(bb, pair_state[p]["qk"][bb], vaug_b, prev, hooks_b)
            # drain the last batch's normalize+projection
            for slot in range(5):
                norm_proj_prev(prev, slot)

    nc.compile()
    return nc


def _get_nc():
    if "nc" not in _CACHE:
        _CACHE["nc"] = _build()
    return _CACHE["nc"]


def _host_prep(x, w_qkv, b_qkv, w_proj, b_proj):
    x = np.asarray(x, dtype=np.float32)
    w_qkv = np.asarray(w_qkv, dtype=np.float32)
    b_qkv = np.asarray(b_qkv, dtype=np.float32)
    w_proj = np.asarray(w_proj, dtype=np.float32)
    b_proj = np.asarray(b_proj, dtype=np.float32)
    bf16 = ml_dtypes.bfloat16

    xt = np.ascontiguousarray(x.transpose(0, 2, 1)).astype(bf16)  # [B, C, N]
    wqk = np.ascontiguousarray(w_qkv[: 2 * C].T).astype(bf16)  # [C, 2C]
    wv_nat = w_qkv[2 * C :]  # [C(hd), C(c)]
    wv = np.zeros((C, VW), dtype=np.float32)
    for h in range(H):
        wv[:, h * 65 : h * 65 + 64] = wv_nat[h * 64 : (h + 1) * 64].T
    wv = wv.astype(bf16)
    bqk = np.ascontiguousarray(b_qkv[: 2 * C].reshape(QK_TILES, 128).T)  # [128, 12]
    wpr = np.ascontiguousarray(w_proj.T).astype(bf16)  # [C, C]
    # v-bias passes through softmax (rows sum to 1): fold into proj bias,
    # then pre-broadcast to [128, C] for the tensor_tensor bias add.
    bp_eff = b_proj + w_proj @ b_qkv[2 * C :]
    bp = np.broadcast_to(bp_eff.reshape(1, C), (128, C)).astype(bf16)
    bp = np.ascontiguousarray(bp)
    return xt, wqk, wv, wpr, bqk, bp


def _run(x, w_qkv, b_qkv, w_proj, b_proj, trace=False, trace_cores=None):
    from concourse.bass_utils import run_bass_kernel_spmd

    xt, wqk, wv, wpr, bqk, bp = _host_prep(x, w_qkv, b_qkv, w_proj, b_proj)
    nc = _get_nc()
    in_maps = []
    for i in range(NCORES):
        in_maps.append(
            {
                "xt": xt[i * BC : (i + 1) * BC],
                "wqk": wqk,
                "wv": wv,
                "wp": wpr,
                "bqk": bqk,
                "bp": bp,
            }
        )
    kwargs = {}
    if trace:
        kwargs = {"trace": True, "trace_cores": trace_cores or [0]}
    res = run_bass_kernel_spmd(nc, in_maps, core_ids=list(range(NCORES)), **kwargs)
    out = np.concatenate([res.results[i]["out"] for i in range(NCORES)], axis=0)
    return out.astype(np.float32), res


def kernel(x, w_qkv, b_qkv, w_proj, b_proj, num_t, num_s):
    assert int(num_t) == NT and int(num_s) == NS
    out, _ = _run(x, w_qkv, b_qkv, w_proj, b_proj)
    return out
